# revision 9
# baseline (speedup 1.0000x reference)
"""2-layer GCN (PyG GCNConv x2, eval mode) on 8 TRN2 NeuronCores, SPMD.

v2: graph-partition data parallel (dst-sharded), aggregation via
dma_gather + one-hot segment-sum matmuls, with:
  - near-zero gather padding: buckets (dst-block x src-range) are padded
    only to the max count over cores (~4%), not to tile multiples; edge
    tiles live on a fixed 128-slot grid and tiles that straddle two
    dst-blocks are aggregated by two masked matmuls (iota window 0..127
    for the primary block, 128..255 for the next block).
  - quarter-chunked AllGathers so gathers start before the full table
    has arrived.
  - both layers share the same edge structure, so IDX/DSTLOC inputs are
    uploaded once and reused.
"""

import numpy as np
import ml_dtypes

import concourse.bass as bass
import concourse.mybir as mybir
import concourse.tile as tile
import concourse.bacc as bacc
from concourse.bass_utils import run_bass_kernel_spmd

F32 = mybir.dt.float32
BF16 = mybir.dt.bfloat16
I16 = mybir.dt.int16
FP8 = mybir.dt.float8e4

P = 128
HID = 128
F_IN = 165
N_NODES = 100000
CORES = 8
NSH = 12500           # real nodes per core shard
NSH_PAD = 12800
QSH = 3200            # quarter of a shard
NRANGE = 4            # src ranges (= quarters of the quarter-major table)
RANGE = CORES * QSH   # 25600 rows per range (int16-addressable)
V_PAD = CORES * NSH_PAD
NB = 100              # dst blocks per core (98 real + 2 empty)
GB = 10               # dst blocks per gather group
NG = NB // GB


def _edge_structure(src, dst):
    """Bucket edges by (dst-core, dst-block, src-range); uniform bucket
    sizes across cores (max over cores)."""
    cdst = np.minimum(dst // NSH, CORES - 1)
    dloc = dst - cdst * NSH              # [0, 12500)
    blk = dloc // P                      # 0..97
    csrc = np.minimum(src // NSH, CORES - 1)
    sloc = src - csrc * NSH
    q = sloc // QSH                      # 0..3
    idx16 = csrc * QSH + (sloc % QSH)    # [0, 25600), int16-safe

    key = ((cdst * NB + blk) * NRANGE + q)
    counts = np.bincount(key, minlength=CORES * NB * NRANGE)
    counts = counts.reshape(CORES, NB, NRANGE)
    bmax = counts.max(axis=0)            # [NB, NRANGE]
    return bmax, key, counts, idx16, dloc


def _layout(bmax):
    """Compile-time slot layout shared by all cores."""
    K_gr = np.zeros((NG, NRANGE), np.int64)
    T_gr = np.zeros((NG, NRANGE), np.int64)
    bucket_start = np.zeros((NB, NRANGE), np.int64)  # slot offset in call
    for g in range(NG):
        for r in range(NRANGE):
            off = 0
            for bl in range(GB):
                b = g * GB + bl
                bucket_start[b, r] = off
                off += int(bmax[b, r])
            K_gr[g, r] = off
            T_gr[g, r] = (off + P - 1) // P
    idx_off = np.zeros((NG, NRANGE), np.int64)
    t_off = np.zeros((NG, NRANGE), np.int64)
    acc_i = acc_t = 0
    for g in range(NG):
        for r in range(NRANGE):
            idx_off[g, r] = acc_i
            t_off[g, r] = acc_t
            acc_i += 8 * int(T_gr[g, r])
            acc_t += int(T_gr[g, r])
    return K_gr, T_gr, bucket_start, idx_off, t_off, int(acc_i), int(acc_t)


def _build_kernel(K_gr, T_gr, bmax, bucket_start, idx_off, t_off, NIDX_COLS, NT_TOT):
    nc = bacc.Bacc("TRN2", target_bir_lowering=False, debug=False,
                   num_devices=CORES)
    T_MAX = int(T_gr.max())

    xT_d = nc.dram_tensor("xT", [F_IN, NSH_PAD], F32, kind="ExternalInput")
    w1_d = nc.dram_tensor("W1", [F_IN, HID], F32, kind="ExternalInput")
    b1t_d = nc.dram_tensor("B1T", [P, HID], F32, kind="ExternalInput")
    w2cb_d = nc.dram_tensor("W2CB", [P, 2 * HID], F32, kind="ExternalInput")
    iota_d = nc.dram_tensor("IOTA", [P, 2 * P], BF16, kind="ExternalInput")
    dinv_d = nc.dram_tensor("DINV", [P, NB], F32, kind="ExternalInput")
    idx_d = nc.dram_tensor("IDX", [P, NIDX_COLS], I16, kind="ExternalInput")
    dstloc_d = nc.dram_tensor("DSTLOC", [P, NT_TOT], BF16,
                              kind="ExternalInput")
    out_d = nc.dram_tensor("OUT", [P, 2 * NB], F32, kind="ExternalOutput")

    KA = min(F_IN, P)
    KB = F_IN - KA

    with tile.TileContext(nc) as tc:
        with (
            tc.tile_pool(name="const", bufs=1) as cpool,
            tc.tile_pool(name="dram", bufs=1, space="DRAM") as dpool,
        ):
            iota_sb = cpool.tile([P, 2 * P], BF16)
            dinv_sb = cpool.tile([P, NB], F32)
            dstloc_sb = cpool.tile([P, NT_TOT], BF16)
            b1t_sb = cpool.tile([P, HID], F32)
            w2cb_sb = cpool.tile([P, 2 * HID], F32)
            outsb = cpool.tile([P, 2 * NB], F32)
            nc.sync.dma_start(out=iota_sb[:], in_=iota_d[:, :])
            nc.sync.dma_start(out=dinv_sb[:], in_=dinv_d[:, :])
            nc.sync.dma_start(out=dstloc_sb[:], in_=dstloc_d[:, :])
            nc.sync.dma_start(out=b1t_sb[:], in_=b1t_d[:, :])
            nc.sync.dma_start(out=w2cb_sb[:], in_=w2cb_d[:, :])

            sh1 = dpool.tile([NSH_PAD, HID], BF16)
            tb1 = dpool.tile([NRANGE * RANGE, HID], BF16)
            sh2 = dpool.tile([NSH_PAD, HID], BF16)
            tb2 = dpool.tile([NRANGE * RANGE, HID], BF16)

            # Phase A: g1 = (xs @ W1) -> sh1 (x pre-scaled by dinv on host)
            with (
                tc.tile_pool(name="mm_w", bufs=1) as wpool,
                tc.tile_pool(name="mm_x", bufs=1) as xpool,
                tc.tile_pool(name="mm_ps", bufs=4, space="PSUM") as pspool,
                tc.tile_pool(name="mm_g", bufs=4) as gpool,
            ):
                w1a = wpool.tile([KA, HID], F32)
                nc.sync.dma_start(out=w1a[:], in_=w1_d[0:KA, :])
                w1b = wpool.tile([KB, HID], F32)
                nc.sync.dma_start(out=w1b[:], in_=w1_d[KA:F_IN, :])
                xta = xpool.tile([KA, NSH_PAD], F32)
                nc.sync.dma_start(out=xta[:], in_=xT_d[0:KA, :])
                xtb = xpool.tile([KB, NSH_PAD], F32)
                nc.sync.dma_start(out=xtb[:], in_=xT_d[KA:F_IN, :])

                for d in range(NSH_PAD // P):
                    ps = pspool.tile([P, HID], F32, space="PSUM", tag="ps_a")
                    sl = slice(d * P, (d + 1) * P)
                    nc.tensor.matmul(out=ps[:], lhsT=xta[:, sl], rhs=w1a[:],
                                     start=True, stop=False)
                    nc.tensor.matmul(out=ps[:], lhsT=xtb[:, sl], rhs=w1b[:],
                                     start=False, stop=True)
                    g1 = gpool.tile([P, HID], BF16, tag="g_a")
                    nc.scalar.copy(out=g1[:], in_=ps[:])
                    nc.sync.dma_start(out=sh1[sl, :], in_=g1[:])

            for q in range(NRANGE):
                nc.gpsimd.collective_compute(
                    "AllGather", mybir.AluOpType.bypass,
                    replica_groups=[list(range(CORES))],
                    ins=[sh1[q * QSH:(q + 1) * QSH, :].opt()],
                    outs=[tb1[q * RANGE:(q + 1) * RANGE, :].opt()],
                )

            # S-block column layout per group: per (g, r): straddle
            # columns then primary-tile columns, all compile-time.
            scol = {}
            SB_COLS = 0
            for g in range(NG):
                off = 0
                for r in range(NRANGE):
                    for bl in range(GB):
                        b = g * GB + bl
                        cnt = int(bmax[b, r])
                        if cnt == 0:
                            continue
                        s0 = int(bucket_start[b, r])
                        s1 = s0 + cnt
                        tp0, t1 = -(-s0 // P), -(-s1 // P)
                        st = (s0 % P) != 0
                        scol[(g, r, b)] = (off, st, tp0, t1)
                        off += (1 if st else 0) + (t1 - tp0)
                SB_COLS = max(SB_COLS, off)

            def agg_pass(table, layer):
                with (
                    tc.tile_pool(name=f"ix{layer}", bufs=2) as ixpool,
                    tc.tile_pool(name=f"st0{layer}", bufs=2) as stpool0,
                    tc.tile_pool(name=f"st{layer}", bufs=1) as stpool,
                    tc.tile_pool(name=f"sb{layer}", bufs=1) as spool,
                    tc.tile_pool(name=f"ps{layer}", bufs=4,
                                 space="PSUM") as pspool,
                    tc.tile_pool(name=f"z{layer}", bufs=4) as zpool,
                ):
                    for g in range(NG):
                        stages = {}
                        for r in range(NRANGE):
                            T = int(T_gr[g, r])
                            if T == 0:
                                continue
                            io = int(idx_off[g, r])
                            ixc = ixpool.tile([P, 8 * T_MAX], I16,
                                              tag=f"ix{r}")
                            nc.sync.dma_start(
                                out=ixc[:, 0:8 * T],
                                in_=idx_d[:, io:io + 8 * T])
                            pool_r = stpool0 if r == 0 else stpool
                            stage = pool_r.tile([P, T_MAX * HID], BF16,
                                                tag=f"st{r}")
                            nc.gpsimd.dma_gather(
                                out_ap=stage[:, 0:T * HID]
                                    .rearrange("p (t j) -> p t j", j=HID),
                                in_ap=table[r * RANGE:(r + 1) * RANGE, :],
                                idxs_ap=ixc[:, 0:8 * T],
                                num_idxs=T * P, num_idxs_reg=T * P,
                                elem_size=HID, single_packet=False)
                            stages[r] = stage
                        # pre-build all one-hots for this group (overlaps
                        # the gathers on DVE)
                        sball = spool.tile([P, SB_COLS * P], FP8, tag="sb")
                        for r in range(NRANGE):
                            to = int(t_off[g, r])
                            for bl in range(GB):
                                b = g * GB + bl
                                if (g, r, b) not in scol:
                                    continue
                                off, st, tp0, t1 = scol[(g, r, b)]
                                co = off + (1 if st else 0)
                                if st:
                                    ts = tp0 - 1
                                    nc.vector.tensor_tensor(
                                        out=sball[:, off * P:(off + 1) * P],
                                        in0=iota_sb[:, P:2 * P],
                                        in1=dstloc_sb[:, to + ts:to + ts + 1]
                                            .to_broadcast([P, P]),
                                        op=mybir.AluOpType.is_equal)
                                if t1 > tp0:
                                    nt = t1 - tp0
                                    nc.vector.tensor_tensor(
                                        out=sball[:, co * P:(co + nt) * P]
                                            .rearrange("p (t j) -> p t j",
                                                       j=P),
                                        in0=iota_sb[:, 0:P].unsqueeze(1)
                                            .to_broadcast([P, nt, P]),
                                        in1=dstloc_sb[:, to + tp0:to + t1]
                                            .unsqueeze(2)
                                            .to_broadcast([P, nt, P]),
                                        op=mybir.AluOpType.is_equal)
                        for bl in range(GB):
                            b = g * GB + bl
                            njobs = 0
                            for r in range(NRANGE):
                                if (g, r, b) in scol:
                                    off, st, tp0, t1 = scol[(g, r, b)]
                                    njobs += (1 if st else 0) + (t1 - tp0)
                            if njobs == 0:
                                continue
                            ps = pspool.tile([P, HID], F32, space="PSUM",
                                             tag="ps")
                            done = 0
                            for r in range(NRANGE):
                                if (g, r, b) not in scol:
                                    continue
                                off, st, tp0, t1 = scol[(g, r, b)]
                                stage = stages[r]
                                co = off + (1 if st else 0)
                                if st:
                                    ts = tp0 - 1
                                    nc.tensor.matmul(
                                        out=ps[:],
                                        lhsT=sball[:, off * P:(off + 1) * P],
                                        rhs=stage[:, ts * HID:(ts + 1) * HID],
                                        start=(done == 0),
                                        stop=(done == njobs - 1))
                                    done += 1
                                for k in range(t1 - tp0):
                                    t = tp0 + k
                                    nc.tensor.matmul(
                                        out=ps[:],
                                        lhsT=sball[:, (co + k) * P:
                                                   (co + k + 1) * P],
                                        rhs=stage[:,
                                                  t * HID:(t + 1) * HID],
                                        start=(done == 0),
                                        stop=(done == njobs - 1))
                                    done += 1
                            yield b, ps, zpool

            # AGG1 + layer-1 tail
            with tc.tile_pool(name="pb1", bufs=4) as pbpool:
                for b, ps, zpool in agg_pass(tb1, 1):
                    z = zpool.tile([P, HID], F32, tag="z1")
                    nc.vector.tensor_scalar(out=z[:], in0=ps[:],
                                            scalar1=dinv_sb[:, b:b + 1],
                                            scalar2=None,
                                            op0=mybir.AluOpType.mult)
                    nc.vector.tensor_tensor(out=z[:], in0=z[:], in1=b1t_sb[:],
                                            op=mybir.AluOpType.add)
                    p_bf = pbpool.tile([P, HID], BF16, tag="pb")
                    nc.scalar.activation(out=p_bf[:], in_=z[:],
                                         func=mybir.ActivationFunctionType.Relu,
                                         scale=dinv_sb[:, b:b + 1])
                    nc.sync.dma_start(out=sh2[b * P:(b + 1) * P, :],
                                      in_=p_bf[:])

            for q in range(NRANGE):
                nc.gpsimd.collective_compute(
                    "AllGather", mybir.AluOpType.bypass,
                    replica_groups=[list(range(CORES))],
                    ins=[sh2[q * QSH:(q + 1) * QSH, :].opt()],
                    outs=[tb2[q * RANGE:(q + 1) * RANGE, :].opt()],
                )

            # AGG2 + layer-2 tail
            nc.vector.memset(outsb[:], 0.0)
            with tc.tile_pool(name="tmp2", bufs=4) as tmppool:
                for b, ps, zpool in agg_pass(tb2, 2):
                    z = zpool.tile([P, HID], F32, tag="z2")
                    nc.vector.tensor_scalar(out=z[:], in0=ps[:],
                                            scalar1=dinv_sb[:, b:b + 1],
                                            scalar2=None,
                                            op0=mybir.AluOpType.mult)
                    for ch in range(2):
                        tmp = tmppool.tile([P, HID], F32, tag="t2")
                        nc.vector.tensor_tensor(
                            out=tmp[:], in0=z[:],
                            in1=w2cb_sb[:, ch * HID:(ch + 1) * HID],
                            op=mybir.AluOpType.mult)
                        nc.vector.tensor_reduce(
                            out=outsb[:, 2 * b + ch:2 * b + ch + 1],
                            in_=tmp[:], axis=mybir.AxisListType.X,
                            op=mybir.AluOpType.add)

            nc.sync.dma_start(out=out_d[:, :], in_=outsb[:])

    nc.compile()
    return nc


def _prep(x, edge_index, W1, b1, W2):
    src = np.asarray(edge_index[0], dtype=np.int64)
    dst = np.asarray(edge_index[1], dtype=np.int64)
    loop = np.arange(N_NODES, dtype=np.int64)
    src = np.concatenate([src, loop])
    dst = np.concatenate([dst, loop])

    deg = np.bincount(dst, minlength=N_NODES)
    dinv = np.where(deg > 0, 1.0 / np.sqrt(deg.astype(np.float64)),
                    0.0).astype(np.float32)

    bmax, key, counts, idx16, dloc = _edge_structure(src, dst)
    (K_gr, T_gr, bucket_start, idx_off, t_off,
     NIDX_COLS, NT_TOT) = _layout(bmax)

    # per-core slot assignment (vectorized-ish)
    order = np.argsort(key, kind="stable")
    k_sorted = key[order]
    starts = np.searchsorted(
        k_sorted, np.arange(CORES * NB * NRANGE + 1))

    idx_all = np.zeros((CORES, P, NIDX_COLS), np.int16)
    dst_all = np.full((CORES, P, NT_TOT), -1000.0, np.float32)

    # primary block per (g, r, tile)
    prim_of_tile = {}
    for g in range(NG):
        for r in range(NRANGE):
            pt = np.zeros(int(T_gr[g, r]), np.int64)
            for bl in range(GB):
                b = g * GB + bl
                s0 = int(bucket_start[b, r])
                s1 = s0 + int(bmax[b, r])
                if s1 == s0:
                    continue
                for t in range(s0 // P, (s1 + P - 1) // P):
                    if s0 <= t * P < s1:
                        pt[t] = b
            prim_of_tile[(g, r)] = pt

    for c in range(CORES):
        for g in range(NG):
            for r in range(NRANGE):
                io = int(idx_off[g, r])
                to = int(t_off[g, r])
                pt = prim_of_tile[(g, r)]
                for bl in range(GB):
                    b = g * GB + bl
                    kk = (c * NB + b) * NRANGE + r
                    s0e, s1e = starts[kk], starts[kk + 1]
                    cnt = s1e - s0e
                    if cnt == 0 and bmax[b, r] == 0:
                        continue
                    e = order[s0e:s1e]
                    bs = int(bucket_start[b, r])
                    # real edges
                    J = bs + np.arange(cnt)
                    idx_all[c, J % 16, io + J // 16] = idx16[e].astype(
                        np.int16)
                    tloc = J // P
                    dst_all[c, J % P, to + tloc] = (
                        dloc[e] - P * pt[tloc]).astype(np.float32)

    # replicate idx rows 0:16 across the 8 partition groups
    for q in range(1, 8):
        idx_all[:, 16 * q:16 * (q + 1), :] = idx_all[:, 0:16, :]

    dinv_pb = np.zeros((CORES, P, NB), np.float32)
    for c in range(CORES):
        n0, n1 = c * NSH, min((c + 1) * NSH, N_NODES)
        loc = np.zeros(NB * P, np.float32)
        loc[: n1 - n0] = dinv[n0:n1]
        dinv_pb[c] = loc.reshape(NB, P).T

    # x pre-scaled by dinv (source-side normalization), quarter-major shard
    xs = np.asarray(x, np.float32) * dinv[:, None]
    xT = np.ascontiguousarray(xs.T)
    b1t = np.tile(np.asarray(b1, np.float32)[None, :], (P, 1))
    w2 = np.asarray(W2, np.float32)
    w2cb = np.zeros((P, 2 * HID), np.float32)
    for ch in range(2):
        w2cb[:, ch * HID:(ch + 1) * HID] = np.tile(w2[:, ch][None, :], (P, 1))
    iota = np.zeros((P, 2 * P), np.float32)
    iota[:, 0:P] = np.tile(np.arange(P, dtype=np.float32)[None, :], (P, 1))
    iota[:, P:2 * P] = iota[:, 0:P] + P
    W1f = np.asarray(W1, np.float32)

    in_maps = []
    for c in range(CORES):
        n0, n1 = c * NSH, min((c + 1) * NSH, N_NODES)
        xtc = np.zeros((F_IN, NSH_PAD), np.float32)
        # shard rows laid out quarter-major: local node u -> quarter
        # u//QSH... within shard it's just contiguous [0,12500) padded.
        xtc[:, : n1 - n0] = xT[:, n0:n1]
        in_maps.append({
            "xT": xtc, "W1": W1f, "B1T": b1t, "W2CB": w2cb,
            "IOTA": iota.astype(ml_dtypes.bfloat16),
            "DINV": dinv_pb[c], "IDX": idx_all[c],
            "DSTLOC": dst_all[c].astype(ml_dtypes.bfloat16),
        })
    return (K_gr, T_gr, bmax, bucket_start, idx_off, t_off,
            NIDX_COLS, NT_TOT, in_maps)


def kernel(x, edge_index, W1, b1, W2, b2):
    import os
    x = np.asarray(x)
    edge_index = np.asarray(edge_index)
    W1 = np.asarray(W1)
    b1 = np.asarray(b1)
    W2 = np.asarray(W2)
    b2 = np.asarray(b2, dtype=np.float32)
    assert x.shape == (N_NODES, F_IN), x.shape

    (K_gr, T_gr, bmax, bucket_start, idx_off, t_off, NIDX_COLS, NT_TOT,
     in_maps) = _prep(x, edge_index, W1, b1, W2)
    nc = _build_kernel(K_gr, T_gr, bmax, bucket_start, idx_off, t_off,
                       NIDX_COLS, NT_TOT)
    trace = bool(int(os.environ.get("GCN_TRACE", "0")))
    try:
        res = run_bass_kernel_spmd(nc, in_maps, core_ids=list(range(CORES)),
                                   trace=trace)
    except Exception:
        if not trace:
            raise
        import traceback
        traceback.print_exc()
        res = run_bass_kernel_spmd(nc, in_maps, core_ids=list(range(CORES)),
                                   trace=False)
    if trace and res.exec_time_ns is not None:
        print(f"HW exec time: {res.exec_time_ns} ns")
    if trace and res.instructions_and_trace is not None:
        print(f"trace path: {res.instructions_and_trace[1]}")

    out = np.zeros((N_NODES, 2), np.float32)
    for c in range(CORES):
        buf = res.results[c]["OUT"]
        arr = buf.reshape(P, NB, 2).transpose(1, 0, 2).reshape(NB * P, 2)
        n0, n1 = c * NSH, min((c + 1) * NSH, N_NODES)
        out[n0:n1] = arr[: n1 - n0]
    return out + b2[None, :]


# revision 10
# speedup vs baseline: 1.3973x; 1.3973x over previous
"""2-layer GCN (PyG GCNConv x2, eval mode) on 8 TRN2 NeuronCores, SPMD.

v2: graph-partition data parallel (dst-sharded), aggregation via
dma_gather + one-hot segment-sum matmuls, with:
  - near-zero gather padding: buckets (dst-block x src-range) are padded
    only to the max count over cores (~4%), not to tile multiples; edge
    tiles live on a fixed 128-slot grid and tiles that straddle two
    dst-blocks are aggregated by two masked matmuls (iota window 0..127
    for the primary block, 128..255 for the next block).
  - quarter-chunked AllGathers so gathers start before the full table
    has arrived.
  - both layers share the same edge structure, so IDX/DSTLOC inputs are
    uploaded once and reused.
"""

import numpy as np
import ml_dtypes

import concourse.bass as bass
import concourse.mybir as mybir
import concourse.tile as tile
import concourse.bacc as bacc
from concourse.bass_utils import run_bass_kernel_spmd

F32 = mybir.dt.float32
BF16 = mybir.dt.bfloat16
I16 = mybir.dt.int16
FP8 = mybir.dt.float8e4

P = 128
HID = 128
F_IN = 165
N_NODES = 100000
CORES = 8
NSH = 12500           # real nodes per core shard
NSH_PAD = 12800
QSH = 3200            # quarter of a shard
NRANGE = 4            # src ranges (= quarters of the quarter-major table)
RANGE = CORES * QSH   # 25600 rows per range (int16-addressable)
V_PAD = CORES * NSH_PAD
NB = 100              # dst blocks per core (98 real + 2 empty)
GB = 10               # dst blocks per gather group
NG = NB // GB


def _edge_structure(src, dst):
    """Bucket edges by (dst-core, dst-block, src-range); uniform bucket
    sizes across cores (max over cores)."""
    cdst = np.minimum(dst // NSH, CORES - 1)
    dloc = dst - cdst * NSH              # [0, 12500)
    blk = dloc // P                      # 0..97
    csrc = np.minimum(src // NSH, CORES - 1)
    sloc = src - csrc * NSH
    q = sloc // QSH                      # 0..3
    idx16 = csrc * QSH + (sloc % QSH)    # [0, 25600), int16-safe

    key = ((cdst * NB + blk) * NRANGE + q)
    counts = np.bincount(key, minlength=CORES * NB * NRANGE)
    counts = counts.reshape(CORES, NB, NRANGE)
    bmax = counts.max(axis=0)            # [NB, NRANGE]
    return bmax, key, counts, idx16, dloc


def _layout(bmax):
    """Compile-time slot layout shared by all cores."""
    K_gr = np.zeros((NG, NRANGE), np.int64)
    T_gr = np.zeros((NG, NRANGE), np.int64)
    bucket_start = np.zeros((NB, NRANGE), np.int64)  # slot offset in call
    for g in range(NG):
        for r in range(NRANGE):
            off = 0
            for bl in range(GB):
                b = g * GB + bl
                bucket_start[b, r] = off
                off += int(bmax[b, r])
            K_gr[g, r] = off
            T_gr[g, r] = (off + P - 1) // P
    idx_off = np.zeros((NG, NRANGE), np.int64)
    t_off = np.zeros((NG, NRANGE), np.int64)
    acc_i = acc_t = 0
    for g in range(NG):
        for r in range(NRANGE):
            idx_off[g, r] = acc_i
            t_off[g, r] = acc_t
            acc_i += 8 * int(T_gr[g, r])
            acc_t += int(T_gr[g, r])
    return K_gr, T_gr, bucket_start, idx_off, t_off, int(acc_i), int(acc_t)


def _build_kernel(K_gr, T_gr, bmax, bucket_start, idx_off, t_off, NIDX_COLS, NT_TOT):
    nc = bacc.Bacc("TRN2", target_bir_lowering=False, debug=False,
                   num_devices=CORES)
    T_MAX = int(T_gr.max())

    xT_d = nc.dram_tensor("xT", [F_IN, NSH_PAD], F32, kind="ExternalInput")
    w1_d = nc.dram_tensor("W1", [F_IN, HID], F32, kind="ExternalInput")
    b1t_d = nc.dram_tensor("B1T", [P, HID], F32, kind="ExternalInput")
    w2cb_d = nc.dram_tensor("W2CB", [P, 2 * HID], F32, kind="ExternalInput")
    iota_d = nc.dram_tensor("IOTA", [P, 2 * P], F32, kind="ExternalInput")
    dinv_d = nc.dram_tensor("DINV", [P, NB], F32, kind="ExternalInput")
    idx_d = nc.dram_tensor("IDX", [P, NIDX_COLS], I16, kind="ExternalInput")
    dstloc_d = nc.dram_tensor("DSTLOC", [P, NT_TOT], F32,
                              kind="ExternalInput")
    out_d = nc.dram_tensor("OUT", [P, 2 * NB], F32, kind="ExternalOutput")

    KA = min(F_IN, P)
    KB = F_IN - KA

    with tile.TileContext(nc) as tc:
        with (
            tc.tile_pool(name="const", bufs=1) as cpool,
            tc.tile_pool(name="dram", bufs=1, space="DRAM") as dpool,
        ):
            iota_sb = cpool.tile([P, 2 * P], F32)
            dinv_sb = cpool.tile([P, NB], F32)
            dstloc_sb = cpool.tile([P, NT_TOT], F32)
            b1t_sb = cpool.tile([P, HID], F32)
            w2cb_sb = cpool.tile([P, 2 * HID], F32)
            outsb = cpool.tile([P, 2 * NB], F32)
            nc.sync.dma_start(out=iota_sb[:], in_=iota_d[:, :])
            nc.sync.dma_start(out=dinv_sb[:], in_=dinv_d[:, :])
            nc.sync.dma_start(out=dstloc_sb[:], in_=dstloc_d[:, :])
            nc.sync.dma_start(out=b1t_sb[:], in_=b1t_d[:, :])
            nc.sync.dma_start(out=w2cb_sb[:], in_=w2cb_d[:, :])

            sh1 = dpool.tile([NSH_PAD, HID], BF16)
            tb1 = dpool.tile([NRANGE * RANGE, HID], BF16)
            sh2 = dpool.tile([NSH_PAD, HID], BF16)
            tb2 = dpool.tile([NRANGE * RANGE, HID], BF16)

            # Phase A: g1 = (xs @ W1) -> sh1 (x pre-scaled by dinv on host)
            with (
                tc.tile_pool(name="mm_w", bufs=1) as wpool,
                tc.tile_pool(name="mm_x", bufs=1) as xpool,
                tc.tile_pool(name="mm_ps", bufs=4, space="PSUM") as pspool,
                tc.tile_pool(name="mm_g", bufs=4) as gpool,
            ):
                w1a = wpool.tile([KA, HID], F32)
                nc.sync.dma_start(out=w1a[:], in_=w1_d[0:KA, :])
                w1b = wpool.tile([KB, HID], F32)
                nc.sync.dma_start(out=w1b[:], in_=w1_d[KA:F_IN, :])
                xta = xpool.tile([KA, NSH_PAD], F32)
                nc.sync.dma_start(out=xta[:], in_=xT_d[0:KA, :])
                xtb = xpool.tile([KB, NSH_PAD], F32)
                nc.sync.dma_start(out=xtb[:], in_=xT_d[KA:F_IN, :])

                for d in range(NSH_PAD // P):
                    ps = pspool.tile([P, HID], F32, space="PSUM", tag="ps_a")
                    sl = slice(d * P, (d + 1) * P)
                    nc.tensor.matmul(out=ps[:], lhsT=xta[:, sl], rhs=w1a[:],
                                     start=True, stop=False)
                    nc.tensor.matmul(out=ps[:], lhsT=xtb[:, sl], rhs=w1b[:],
                                     start=False, stop=True)
                    g1 = gpool.tile([P, HID], BF16, tag="g_a")
                    nc.scalar.copy(out=g1[:], in_=ps[:])
                    nc.sync.dma_start(out=sh1[sl, :], in_=g1[:])

            for q in range(NRANGE):
                nc.gpsimd.collective_compute(
                    "AllGather", mybir.AluOpType.bypass,
                    replica_groups=[list(range(CORES))],
                    ins=[sh1[q * QSH:(q + 1) * QSH, :].opt()],
                    outs=[tb1[q * RANGE:(q + 1) * RANGE, :].opt()],
                )

            # S-block column layout per group: per (g, r): straddle
            # columns then primary-tile columns, all compile-time.
            scol = {}
            SB_COLS = 0
            for g in range(NG):
                off = 0
                for r in range(NRANGE):
                    for bl in range(GB):
                        b = g * GB + bl
                        cnt = int(bmax[b, r])
                        if cnt == 0:
                            continue
                        s0 = int(bucket_start[b, r])
                        s1 = s0 + cnt
                        tp0, t1 = -(-s0 // P), -(-s1 // P)
                        st = (s0 % P) != 0
                        scol[(g, r, b)] = (off, st, tp0, t1)
                        off += (1 if st else 0) + (t1 - tp0)
                SB_COLS = max(SB_COLS, off)

            def agg_pass(table, layer):
                with (
                    tc.tile_pool(name=f"ix{layer}", bufs=2) as ixpool,
                    tc.tile_pool(name=f"st{layer}", bufs=2) as stpool,
                    tc.tile_pool(name=f"sb{layer}", bufs=1) as spool,
                    tc.tile_pool(name=f"ps{layer}", bufs=4,
                                 space="PSUM") as pspool,
                    tc.tile_pool(name=f"z{layer}", bufs=4) as zpool,
                ):
                    for g in range(NG):
                        stages = {}
                        for r in range(NRANGE):
                            T = int(T_gr[g, r])
                            if T == 0:
                                continue
                            io = int(idx_off[g, r])
                            ixc = ixpool.tile([P, 8 * T_MAX], I16,
                                              tag=f"ix{r}")
                            nc.scalar.dma_start(
                                out=ixc[:, 0:8 * T],
                                in_=idx_d[:, io:io + 8 * T])
                            stage = stpool.tile([P, T_MAX * HID], BF16,
                                                tag=f"st{r}")
                            nc.gpsimd.dma_gather(
                                out_ap=stage[:, 0:T * HID]
                                    .rearrange("p (t j) -> p t j", j=HID),
                                in_ap=table[r * RANGE:(r + 1) * RANGE, :],
                                idxs_ap=ixc[:, 0:8 * T],
                                num_idxs=T * P, num_idxs_reg=T * P,
                                elem_size=HID, single_packet=False)
                            stages[r] = stage
                        # pre-build all one-hots for this group (overlaps
                        # the gathers on DVE)
                        sball = spool.tile([P, SB_COLS * P], FP8, tag="sb")
                        for r in range(NRANGE):
                            to = int(t_off[g, r])
                            for bl in range(GB):
                                b = g * GB + bl
                                if (g, r, b) not in scol:
                                    continue
                                off, st, tp0, t1 = scol[(g, r, b)]
                                co = off + (1 if st else 0)
                                if st:
                                    ts = tp0 - 1
                                    nc.vector.tensor_tensor(
                                        out=sball[:, off * P:(off + 1) * P],
                                        in0=iota_sb[:, P:2 * P],
                                        in1=dstloc_sb[:, to + ts:to + ts + 1]
                                            .to_broadcast([P, P]),
                                        op=mybir.AluOpType.is_equal)
                                if t1 > tp0:
                                    nt = t1 - tp0
                                    nc.vector.tensor_tensor(
                                        out=sball[:, co * P:(co + nt) * P]
                                            .rearrange("p (t j) -> p t j",
                                                       j=P),
                                        in0=iota_sb[:, 0:P].unsqueeze(1)
                                            .to_broadcast([P, nt, P]),
                                        in1=dstloc_sb[:, to + tp0:to + t1]
                                            .unsqueeze(2)
                                            .to_broadcast([P, nt, P]),
                                        op=mybir.AluOpType.is_equal)
                        for bl in range(GB):
                            b = g * GB + bl
                            njobs = 0
                            for r in range(NRANGE):
                                if (g, r, b) in scol:
                                    off, st, tp0, t1 = scol[(g, r, b)]
                                    njobs += (1 if st else 0) + (t1 - tp0)
                            if njobs == 0:
                                continue
                            ps = pspool.tile([P, HID], F32, space="PSUM",
                                             tag="ps")
                            done = 0
                            for r in range(NRANGE):
                                if (g, r, b) not in scol:
                                    continue
                                off, st, tp0, t1 = scol[(g, r, b)]
                                stage = stages[r]
                                co = off + (1 if st else 0)
                                if st:
                                    ts = tp0 - 1
                                    nc.tensor.matmul(
                                        out=ps[:],
                                        lhsT=sball[:, off * P:(off + 1) * P],
                                        rhs=stage[:, ts * HID:(ts + 1) * HID],
                                        start=(done == 0),
                                        stop=(done == njobs - 1))
                                    done += 1
                                for k in range(t1 - tp0):
                                    t = tp0 + k
                                    nc.tensor.matmul(
                                        out=ps[:],
                                        lhsT=sball[:, (co + k) * P:
                                                   (co + k + 1) * P],
                                        rhs=stage[:,
                                                  t * HID:(t + 1) * HID],
                                        start=(done == 0),
                                        stop=(done == njobs - 1))
                                    done += 1
                            yield b, ps, zpool

            # AGG1 + layer-1 tail
            with tc.tile_pool(name="pb1", bufs=4) as pbpool:
                for b, ps, zpool in agg_pass(tb1, 1):
                    z = zpool.tile([P, HID], F32, tag="z1")
                    nc.vector.tensor_scalar(out=z[:], in0=ps[:],
                                            scalar1=dinv_sb[:, b:b + 1],
                                            scalar2=None,
                                            op0=mybir.AluOpType.mult)
                    nc.vector.tensor_tensor(out=z[:], in0=z[:], in1=b1t_sb[:],
                                            op=mybir.AluOpType.add)
                    p_bf = pbpool.tile([P, HID], BF16, tag="pb")
                    nc.scalar.activation(out=p_bf[:], in_=z[:],
                                         func=mybir.ActivationFunctionType.Relu,
                                         scale=dinv_sb[:, b:b + 1])
                    nc.sync.dma_start(out=sh2[b * P:(b + 1) * P, :],
                                      in_=p_bf[:])

            for q in range(NRANGE):
                nc.gpsimd.collective_compute(
                    "AllGather", mybir.AluOpType.bypass,
                    replica_groups=[list(range(CORES))],
                    ins=[sh2[q * QSH:(q + 1) * QSH, :].opt()],
                    outs=[tb2[q * RANGE:(q + 1) * RANGE, :].opt()],
                )

            # AGG2 + layer-2 tail
            nc.vector.memset(outsb[:], 0.0)
            with tc.tile_pool(name="tmp2", bufs=4) as tmppool:
                for b, ps, zpool in agg_pass(tb2, 2):
                    z = zpool.tile([P, HID], F32, tag="z2")
                    nc.vector.tensor_scalar(out=z[:], in0=ps[:],
                                            scalar1=dinv_sb[:, b:b + 1],
                                            scalar2=None,
                                            op0=mybir.AluOpType.mult)
                    for ch in range(2):
                        tmp = tmppool.tile([P, HID], F32, tag="t2")
                        nc.vector.tensor_tensor(
                            out=tmp[:], in0=z[:],
                            in1=w2cb_sb[:, ch * HID:(ch + 1) * HID],
                            op=mybir.AluOpType.mult)
                        nc.vector.tensor_reduce(
                            out=outsb[:, 2 * b + ch:2 * b + ch + 1],
                            in_=tmp[:], axis=mybir.AxisListType.X,
                            op=mybir.AluOpType.add)

            nc.sync.dma_start(out=out_d[:, :], in_=outsb[:])

    nc.compile()
    return nc


def _prep(x, edge_index, W1, b1, W2):
    src = np.asarray(edge_index[0], dtype=np.int64)
    dst = np.asarray(edge_index[1], dtype=np.int64)
    loop = np.arange(N_NODES, dtype=np.int64)
    src = np.concatenate([src, loop])
    dst = np.concatenate([dst, loop])

    deg = np.bincount(dst, minlength=N_NODES)
    dinv = np.where(deg > 0, 1.0 / np.sqrt(deg.astype(np.float64)),
                    0.0).astype(np.float32)

    bmax, key, counts, idx16, dloc = _edge_structure(src, dst)
    (K_gr, T_gr, bucket_start, idx_off, t_off,
     NIDX_COLS, NT_TOT) = _layout(bmax)

    # per-core slot assignment (vectorized-ish)
    order = np.argsort(key, kind="stable")
    k_sorted = key[order]
    starts = np.searchsorted(
        k_sorted, np.arange(CORES * NB * NRANGE + 1))

    idx_all = np.zeros((CORES, P, NIDX_COLS), np.int16)
    dst_all = np.full((CORES, P, NT_TOT), -1000.0, np.float32)

    # primary block per (g, r, tile)
    prim_of_tile = {}
    for g in range(NG):
        for r in range(NRANGE):
            pt = np.zeros(int(T_gr[g, r]), np.int64)
            for bl in range(GB):
                b = g * GB + bl
                s0 = int(bucket_start[b, r])
                s1 = s0 + int(bmax[b, r])
                if s1 == s0:
                    continue
                for t in range(s0 // P, (s1 + P - 1) // P):
                    if s0 <= t * P < s1:
                        pt[t] = b
            prim_of_tile[(g, r)] = pt

    for c in range(CORES):
        for g in range(NG):
            for r in range(NRANGE):
                io = int(idx_off[g, r])
                to = int(t_off[g, r])
                pt = prim_of_tile[(g, r)]
                for bl in range(GB):
                    b = g * GB + bl
                    kk = (c * NB + b) * NRANGE + r
                    s0e, s1e = starts[kk], starts[kk + 1]
                    cnt = s1e - s0e
                    if cnt == 0 and bmax[b, r] == 0:
                        continue
                    e = order[s0e:s1e]
                    bs = int(bucket_start[b, r])
                    # real edges
                    J = bs + np.arange(cnt)
                    idx_all[c, J % 16, io + J // 16] = idx16[e].astype(
                        np.int16)
                    tloc = J // P
                    dst_all[c, J % P, to + tloc] = (
                        dloc[e] - P * pt[tloc]).astype(np.float32)

    # replicate idx rows 0:16 across the 8 partition groups
    for q in range(1, 8):
        idx_all[:, 16 * q:16 * (q + 1), :] = idx_all[:, 0:16, :]

    dinv_pb = np.zeros((CORES, P, NB), np.float32)
    for c in range(CORES):
        n0, n1 = c * NSH, min((c + 1) * NSH, N_NODES)
        loc = np.zeros(NB * P, np.float32)
        loc[: n1 - n0] = dinv[n0:n1]
        dinv_pb[c] = loc.reshape(NB, P).T

    # x pre-scaled by dinv (source-side normalization), quarter-major shard
    xs = np.asarray(x, np.float32) * dinv[:, None]
    xT = np.ascontiguousarray(xs.T)
    b1t = np.tile(np.asarray(b1, np.float32)[None, :], (P, 1))
    w2 = np.asarray(W2, np.float32)
    w2cb = np.zeros((P, 2 * HID), np.float32)
    for ch in range(2):
        w2cb[:, ch * HID:(ch + 1) * HID] = np.tile(w2[:, ch][None, :], (P, 1))
    iota = np.zeros((P, 2 * P), np.float32)
    iota[:, 0:P] = np.tile(np.arange(P, dtype=np.float32)[None, :], (P, 1))
    iota[:, P:2 * P] = iota[:, 0:P] + P
    W1f = np.asarray(W1, np.float32)

    in_maps = []
    for c in range(CORES):
        n0, n1 = c * NSH, min((c + 1) * NSH, N_NODES)
        xtc = np.zeros((F_IN, NSH_PAD), np.float32)
        # shard rows laid out quarter-major: local node u -> quarter
        # u//QSH... within shard it's just contiguous [0,12500) padded.
        xtc[:, : n1 - n0] = xT[:, n0:n1]
        in_maps.append({
            "xT": xtc, "W1": W1f, "B1T": b1t, "W2CB": w2cb,
            "IOTA": iota,
            "DINV": dinv_pb[c], "IDX": idx_all[c],
            "DSTLOC": dst_all[c],
        })
    return (K_gr, T_gr, bmax, bucket_start, idx_off, t_off,
            NIDX_COLS, NT_TOT, in_maps)


def kernel(x, edge_index, W1, b1, W2, b2):
    import os
    x = np.asarray(x)
    edge_index = np.asarray(edge_index)
    W1 = np.asarray(W1)
    b1 = np.asarray(b1)
    W2 = np.asarray(W2)
    b2 = np.asarray(b2, dtype=np.float32)
    assert x.shape == (N_NODES, F_IN), x.shape

    (K_gr, T_gr, bmax, bucket_start, idx_off, t_off, NIDX_COLS, NT_TOT,
     in_maps) = _prep(x, edge_index, W1, b1, W2)
    nc = _build_kernel(K_gr, T_gr, bmax, bucket_start, idx_off, t_off,
                       NIDX_COLS, NT_TOT)
    trace = bool(int(os.environ.get("GCN_TRACE", "0")))
    try:
        res = run_bass_kernel_spmd(nc, in_maps, core_ids=list(range(CORES)),
                                   trace=trace)
    except Exception:
        if not trace:
            raise
        import traceback
        traceback.print_exc()
        res = run_bass_kernel_spmd(nc, in_maps, core_ids=list(range(CORES)),
                                   trace=False)
    if trace and res.exec_time_ns is not None:
        print(f"HW exec time: {res.exec_time_ns} ns")
    if trace and res.instructions_and_trace is not None:
        print(f"trace path: {res.instructions_and_trace[1]}")

    out = np.zeros((N_NODES, 2), np.float32)
    for c in range(CORES):
        buf = res.results[c]["OUT"]
        arr = buf.reshape(P, NB, 2).transpose(1, 0, 2).reshape(NB * P, 2)
        n0, n1 = c * NSH, min((c + 1) * NSH, N_NODES)
        out[n0:n1] = arr[: n1 - n0]
    return out + b2[None, :]
